# revision 1
# baseline (speedup 1.0000x reference)
"""Grouped (MoE-style) linear on 8 trn2 NeuronCores.

out[t] = hidden_states[t] @ weight[g(t)], where token t belongs to group g iff
offsets[g-1] <= t < offsets[g] (searchsorted right semantics; tokens at or past
offsets[-1] get zero output).

Strategy: expert-parallel. Core g owns weight[g] and the contiguous token run
of group g. Routing is done host-side (offsets are host data), each core runs
an identical Bass program: a [P_pad, 1024] x [1024, 1024] matmul tiled as
128-token blocks, contraction in 8 chunks of 128, PSUM-accumulated, fp32r
matmul (full PE rate; ~1.4e-4 relmax vs fp64 measured for this distribution).

Host packs per-core inputs so every DMA lands with >=4KB contiguous runs:
  xt[tb, p, k, tok] = X_g[tb*128 + tok, k*128 + p]   (transposed token block)
  w[p, k, n]        = W_g[k*128 + p, n]
"""
import numpy as np

import concourse.bass as bass
import concourse.tile as tile
from concourse import bacc, mybir
from concourse.bass_utils import run_bass_kernel_spmd

GROUPS = 8
TOKENS = 16384
IN_F = 1024
OUT_F = 1024
KCH = IN_F // 128  # contraction chunks




def build(ntb: int) -> bass.Bass:
    """One core's program: ntb 128-token blocks through a 1024x1024 expert."""
    f32 = mybir.dt.float32
    f32r = mybir.dt.float32r
    nc = bacc.Bacc()
    xt_d = nc.dram_tensor("xt", [ntb, 128, KCH, 128], f32r, kind="ExternalInput")
    w_d = nc.dram_tensor("w", [128, KCH, OUT_F], f32r, kind="ExternalInput")
    out_d = nc.dram_tensor("out", [ntb * 128, OUT_F], f32, kind="ExternalOutput")

    with tile.TileContext(nc) as tc:
        with (
            tc.tile_pool(name="wp", bufs=1) as wp,
            tc.tile_pool(name="xp", bufs=ntb) as xp,
            tc.tile_pool(name="op", bufs=4) as op,
            tc.tile_pool(name="ps", bufs=4, space="PSUM") as psp,
        ):
            wt = wp.tile([128, KCH, OUT_F], f32r)
            xts = []
            # startup: the first matmul needs only xt0's k=0 chunk (64KB) and
            # W chunk 0's first half (256KB); land those first, then the
            # next 3 token blocks (to keep 4 PSUM groups runnable while the
            # remaining W chunks stream in), then W k=1..7.
            xt0 = xp.tile([128, KCH, 128], f32r, tag="xt")
            nc.sync.dma_start(out=xt0[:, 0, :], in_=xt_d[0, :, 0, :])
            nc.scalar.dma_start(out=wt[:, 0, 0:512], in_=w_d[:, 0, 0:512])
            nc.sync.dma_start(out=xt0[:, 1:, :], in_=xt_d[0, :, 1:, :])
            nc.scalar.dma_start(out=wt[:, 0, 512:], in_=w_d[:, 0, 512:])
            xts.append(xt0)
            for t in range(1, min(4, ntb)):
                xtn = xp.tile([128, KCH, 128], f32r, tag="xt")
                nc.sync.dma_start(out=xtn[:], in_=xt_d[t])
                xts.append(xtn)
            for k in range(1, KCH):
                nc.sync.dma_start(out=wt[:, k, :], in_=w_d[:, k, :])
            for tb in range(ntb):
                if tb < len(xts):
                    xt = xts[tb]
                else:
                    xt = xp.tile([128, KCH, 128], f32r, tag="xt")
                    nc.sync.dma_start(out=xt[:], in_=xt_d[tb])
                ps = psp.tile([128, OUT_F], f32)
                if tb == 0:
                    # PE p-state warmup: re-issue the first matmul; start=True
                    # resets the bank each time so only the last one counts.
                    for _ in range(12):
                        nc.tensor.matmul(ps[:, 0:512], xt[:, 0, :],
                                         wt[:, 0, 0:512], start=True,
                                         stop=True, skip_group_check=True)
                for k in range(KCH):
                    for nb in range(OUT_F // 512):
                        nc.tensor.matmul(
                            ps[:, nb * 512:(nb + 1) * 512],
                            xt[:, k, :],
                            wt[:, k, nb * 512:(nb + 1) * 512],
                            start=(k == 0),
                            stop=(k == KCH - 1),
                        )
                ot = op.tile([128, OUT_F], f32)
                nc.scalar.copy(ot[:, 0:512], ps[:, 0:512])
                nc.vector.tensor_copy(ot[:, 512:1024], ps[:, 512:1024])
                if tb == ntb - 1:
                    nc.scalar.dma_start(out=out_d[tb * 128:(tb + 1) * 128, 0:512],
                                        in_=ot[:, 0:512])
                    nc.sync.dma_start(out=out_d[tb * 128:(tb + 1) * 128, 512:1024],
                                      in_=ot[:, 512:1024])
                else:
                    nc.scalar.dma_start(out=out_d[tb * 128:(tb + 1) * 128, :], in_=ot[:])
    nc.compile()
    return nc


def _pack_core(x_slice: np.ndarray, w_g: np.ndarray, ntb: int):
    n = x_slice.shape[0]
    xp = np.zeros((ntb * 128, IN_F), dtype=np.float32)
    xp[:n] = x_slice
    xt = np.ascontiguousarray(
        xp.reshape(ntb, 128, KCH, 128).transpose(0, 3, 2, 1)
    )
    wt = np.ascontiguousarray(w_g.reshape(KCH, 128, OUT_F).transpose(1, 0, 2))
    return xt, wt


def kernel(hidden_states: np.ndarray, weight: np.ndarray, offsets: np.ndarray,
           _trace: bool = False):
    hs = np.ascontiguousarray(hidden_states, dtype=np.float32)
    w = np.ascontiguousarray(weight, dtype=np.float32)
    off = np.asarray(offsets).astype(np.int64)

    ends = np.clip(off, 0, TOKENS)
    starts = np.concatenate(([0], ends[:-1]))
    starts = np.minimum(starts, ends)
    ns = ends - starts

    ntb = max(1, int(-(-ns.max() // 128)))
    nc = build(ntb)

    in_maps = []
    for g in range(GROUPS):
        xt, wt = _pack_core(hs[starts[g]:ends[g]], w[g], ntb)
        in_maps.append({"xt": xt, "w": wt})

    res = run_bass_kernel_spmd(nc, in_maps, list(range(GROUPS)), trace=_trace)

    out = np.zeros((TOKENS, OUT_F), dtype=np.float32)
    for g in range(GROUPS):
        if ns[g] > 0:
            out[starts[g]:ends[g]] = res.results[g]["out"][:ns[g]]
    if _trace:
        return out, res
    return out

